# revision 1
# baseline (speedup 1.0000x reference)
"""AdaptiveConv Trainium2 kernel — 8-core SPMD, batch-sharded.

Per full batch:
  x [16, 256, 64, 64] f32, w [16, 512] f32,
  filter_bank [8, 256, 256, 3, 3], dense_fw (512->8), dense_mod (512->256).
  fbw = softmax(w @ Wfw + bfw)                  [16, 8]
  filters = einsum('bfchw,nb->nfchw', bank, fbw)
  filters *= (w @ Wmod + bmod + 1)[n, f]
  norm[n,kh,kw] = sqrt(max(sum_{f,c} filters^2, 1e-8)); filters /= norm
  out[n] = conv2d_same(x[n], filters[n])        [16, 256, 64, 64]

Sharding: batch N=16 over 8 cores (2 samples each); small params + the
filter bank replicated. The conv is an implicit GEMM in padded-flat
pixel coordinates: x lives in SBUF as [c=128, 66*66] zero-halo tiles,
so each of the 18 accumulating matmuls (2 c-tiles x 9 taps) per output
chunk streams a fully contiguous 512-wide rhs (full PE rate). Output
columns 64,65 of each row are garbage and dropped by the strided
output DMA.

Filter mixing uses DVE fast modes (tensor_scalar 4x + tensor_tensor
2x in bf16) instead of 1x scalar_tensor_tensor. Dummy paced matmuls
keep the PE HAM clock warm while sample 0's filters are being mixed.

Host-side work is layout-only: batch slicing, a transpose of
filter_bank to [b, c, tap, f] (+ optional bf16 cast), and w to
[p, ko, n] so every DMA is a single contiguous descriptor run.
"""

import os
import sys

import numpy as np

if "/opt/trn_rl_repo" not in sys.path:
    sys.path.insert(0, "/opt/trn_rl_repo")

import concourse.bacc as bacc_mod
import concourse.mybir as mybir
import concourse.tile as tile
from concourse.bass_utils import run_bass_kernel_spmd

N_CORES = 8
NS = 2            # samples per core
C = 256
F = 256
H = W = 64
KK = 3
TAPS = KK * KK    # 9
NF = 8
WD = 512
P = 128
CT = C // P       # 2 c tiles
FT = F // P       # 2 f tiles
KO = WD // P      # 4 contraction tiles for the dense layers
HP = H + 2        # 66
WP = W + 2        # 66
XL = HP * WP      # 4356 padded pixels
XLPAD = XL + 8    # tile length incl. slack for tap-offset over-reads
OPIX = H * WP     # 4224 output pixels in padded-w coords
CHUNKS = [(i * 512, 512) for i in range(8)] + [(4096, OPIX - 4096)]
MIX_CHUNKS = [(i * 512, 512) for i in range(4)] + [(2048, TAPS * F - 2048)]
EPS = 1e-8

USE_BF16 = os.environ.get("KERNEL_F32", "") != "1"
WARM_MM = int(os.environ.get("KERNEL_WARM_MM", "2"))  # warmup MMs per bank DMA

LAST = None       # BassKernelResults of the most recent run (for test.py)


def _build():
    f32 = mybir.dt.float32
    cdt = mybir.dt.bfloat16 if USE_BF16 else f32

    nc = bacc_mod.Bacc()
    x_d = nc.declare_dram_parameter("xp", [NS, CT, P, XLPAD], cdt, isOutput=False)
    w_d = nc.declare_dram_parameter("wv_t", [P, KO, NS], f32, isOutput=False)
    bank_d = nc.declare_dram_parameter("bank_t", [NF, C, TAPS, F], cdt,
                                       isOutput=False)
    fww_d = nc.declare_dram_parameter("fw_w", [WD, NF], cdt, isOutput=False)
    fwb_d = nc.declare_dram_parameter("fw_b", [NF], f32, isOutput=False)
    mdw_d = nc.declare_dram_parameter("md_w", [WD, F], cdt, isOutput=False)
    mdb_d = nc.declare_dram_parameter("md_b", [F], f32, isOutput=False)
    ident_d = nc.declare_dram_parameter("ident", [P, P], cdt, isOutput=False)
    out_d = nc.declare_dram_parameter("out", [NS, F, H, W], f32, isOutput=True)

    mm = mybir.AluOpType.mult
    aa = mybir.AluOpType.add
    ACT = mybir.ActivationFunctionType

    with tile.TileContext(nc) as tc, \
         tc.tile_pool(name="const", bufs=1) as const_p, \
         tc.tile_pool(name="small", bufs=2) as small_p, \
         tc.tile_pool(name="bcast", bufs=2) as bc_p, \
         tc.tile_pool(name="dscr", bufs=1, space="DRAM") as dram_p, \
         tc.tile_pool(name="xpad", bufs=(4 if USE_BF16 else 3)) as xpad_p, \
         tc.tile_pool(name="bank", bufs=(8 if USE_BF16 else 2)) as bank_p, \
         tc.tile_pool(name="tmp", bufs=3) as tmp_p, \
         tc.tile_pool(name="diag", bufs=8) as diag_p, \
         tc.tile_pool(name="mix", bufs=2) as mix_p, \
         tc.tile_pool(name="filtb", bufs=3) as filtb_p, \
         tc.tile_pool(name="outs", bufs=2) as out_p, \
         tc.tile_pool(name="pscv", bufs=6, space="PSUM") as ps_conv, \
         tc.tile_pool(name="pssm", bufs=1, space="PSUM") as ps_small:

        # preload ACT tables (Exp/Square/Sqrt) with a self-contained tile
        # so the loads never join the softmax dependency chain
        tblw = small_p.tile([P, 2], f32, name="tblw")
        nc.vector.memset(tblw, 0.5)
        nc.scalar.activation(tblw, tblw, ACT.Exp)
        nc.scalar.activation(tblw, tblw, ACT.Square)
        nc.scalar.activation(tblw, tblw, ACT.Sqrt)

        # ---- small parameter loads -------------------------------------
        wt = const_p.tile([P, KO, NS], f32)
        nc.sync.dma_start(wt, w_d[:, :, :])
        fww = const_p.tile([P, KO, NF], cdt)
        nc.sync.dma_start(fww, fww_d[:, :].rearrange("(ko p) f -> p ko f", p=P))
        mdw = const_p.tile([P, KO, F], cdt)
        nc.sync.dma_start(mdw, mdw_d[:, :].rearrange("(ko p) f -> p ko f", p=P))
        fwb_bc = const_p.tile([P, NF], f32)
        nc.sync.dma_start(fwb_bc, fwb_d[:][None, :].to_broadcast((P, NF)))
        mdb_bc = const_p.tile([P, F], f32)
        nc.sync.dma_start(mdb_bc, mdb_d[:][None, :].to_broadcast((P, F)))


        # ---- dense heads, replicated across all 128 partitions ---------
        # lhsT columns all equal w[n], so every psum partition holds the
        # same dense result; softmax runs redundantly per partition and
        # fbw/w1 land pre-broadcast with no DRAM bounce.
        fbw_bc = []
        w1rep = []
        for n in range(NS):
            wrep = bc_p.tile([P, KO, P], cdt, tag="wrep")
            nc.vector.tensor_copy(
                wrep, wt[:, :, n][:, :, None].to_broadcast((P, KO, P)))
            ps_l = ps_small.tile([P, NF], f32, tag="sm")
            for ko in range(KO):
                nc.tensor.matmul(ps_l, lhsT=wrep[:, ko, :], rhs=fww[:, ko, :],
                                 start=(ko == 0), stop=(ko == KO - 1))
            logits = small_p.tile([P, NF], f32, tag="logits")
            nc.vector.tensor_tensor(logits, ps_l, fwb_bc, aa)
            # softmax denominator cancels through the per-tap demod norm
            # (filters/||filters|| is invariant to any per-sample scalar),
            # so raw exp(logits) works as mixing weights.
            fb = bc_p.tile([P, NF], f32, tag="fbw_bc")
            nc.scalar.activation(fb, logits, ACT.Exp)
            fbw_bc.append(fb)

            ps_sc = ps_small.tile([P, F], f32, tag="sm")
            for ko in range(KO):
                nc.tensor.matmul(ps_sc, lhsT=wrep[:, ko, :], rhs=mdw[:, ko, :],
                                 start=(ko == 0), stop=(ko == KO - 1))
            w1 = bc_p.tile([P, F], cdt, tag="w1_bc")
            nc.vector.scalar_tensor_tensor(w1, ps_sc, 1.0, mdb_bc, aa, aa)
            w1r = bc_p.tile([P, TAPS, F], cdt, tag="w1rep")
            nc.vector.tensor_copy(
                w1r, w1[:, None, :].to_broadcast((P, TAPS, F)))
            w1rep.append(w1r)

        # psum for HAM warmup matmuls
        ps_warm = ps_small.tile([P, 512], f32, tag="warm")
        ones_sb = const_p.tile([P, P], f32)
        nc.vector.memset(ones_sb, 1.0)
        ident = const_p.tile([P, P], cdt)
        nc.sync.dma_start(ident, ident_d[:, :])

        # ---- per-sample: pad x, mix+modulate+demod filters, conv -------
        for n in range(NS):
            # host-pre-padded input in flat coords, [c=128, 66*66+slack]
            xpt = []
            for ct in range(CT):
                xp = xpad_p.tile([P, XLPAD], cdt, tag="xpad")
                nc.scalar.dma_start(xp, x_d[n, ct, :, :])
                xpt.append(xp)

            # mix bank with fbw, then modulate by w1.
            # Sample 0 is on the critical path: accumulate fbw_b * bank_b on
            # the otherwise-idle PE as matmuls with diagonal fbw_b*I weights
            # (f32 PSUM accumulate). Sample 1 mixes on DVE under sample 0's
            # conv.
            acc = []
            q0 = []
            if n == 0:
                fbwI = []
                for b in range(NF):
                    dg = diag_p.tile([P, P], cdt, tag="diag")
                    nc.vector.tensor_scalar_mul(dg, ident,
                                                fbw_bc[n][:, b:b + 1])
                    fbwI.append(dg)
                    if WARM_MM:
                        for _ in range(2):
                            nc.tensor.matmul(ps_warm[:, :P], lhsT=dg, rhs=dg,
                                             start=True, stop=True)
                for ct in range(CT):
                    bks = []
                    for b in range(NF):
                        bk = bank_p.tile([P, TAPS, F], cdt, tag=f"bk{ct}")
                        nc.sync.dma_start(bk,
                                          bank_d[b, ct * P:(ct + 1) * P, :, :])
                        bks.append(bk)
                        if ct == 0 and WARM_MM:
                            for _ in range(WARM_MM):
                                nc.tensor.matmul(ps_warm,
                                                 lhsT=bk[:, 0, 0:P],
                                                 rhs=bk[:, 0:2, :],
                                                 start=True, stop=True)
                    a = mix_p.tile([P, TAPS, F], cdt, tag="acc")
                    af = a.rearrange("p t f -> p (t f)")
                    scr = tmp_p.tile([P, TAPS, F], cdt, tag="tmp")
                    scrf = scr.rearrange("p t f -> p (t f)")
                    w1f = w1rep[n].rearrange("p t f -> p (t f)")
                    qt = small_p.tile([P, TAPS], f32, tag="q")
                    for ci, (off, csz) in enumerate(MIX_CHUNKS):
                        ps = ps_conv.tile([P, 512], f32, tag="cv")
                        for b in range(NF):
                            bf = bks[b].rearrange("p t f -> p (t f)")
                            nc.tensor.matmul(ps[:, :csz], lhsT=fbwI[b],
                                             rhs=bf[:, off:off + csz],
                                             start=(b == 0),
                                             stop=(b == NF - 1))
                        nc.vector.tensor_tensor(af[:, off:off + csz],
                                                ps[:, :csz],
                                                w1f[:, off:off + csz], mm)
                        nc.scalar.activation(scrf[:, off:off + csz],
                                             af[:, off:off + csz], ACT.Square)
                        if ci == 3:
                            # taps 0-7 reduce overlaps the final mix chunk
                            nc.vector.tensor_reduce(
                                qt[:, 0:8], scr[:, 0:8, :],
                                axis=mybir.AxisListType.X, op=aa)
                    nc.vector.tensor_reduce(qt[:, 8:9], scr[:, 8:9, :],
                                            axis=mybir.AxisListType.X, op=aa)
                    q0.append(qt)
                    acc.append(a)
            else:
                for ct in range(CT):
                    a = mix_p.tile([P, TAPS, F], cdt, tag="acc")
                    for b in range(NF):
                        bk = bank_p.tile([P, TAPS, F], cdt, tag=f"bk{ct}")
                        nc.sync.dma_start(bk,
                                          bank_d[b, ct * P:(ct + 1) * P, :, :])
                        if b == 0:
                            nc.vector.tensor_scalar_mul(a, bk,
                                                        fbw_bc[n][:, 0:1])
                        else:
                            t = tmp_p.tile([P, TAPS, F], cdt, tag="tmp")
                            if b in (5, 7):
                                nc.scalar.activation(
                                    t, bk, ACT.Copy,
                                    scale=fbw_bc[n][:, b:b + 1])
                            else:
                                nc.vector.tensor_scalar_mul(
                                    t, bk, fbw_bc[n][:, b:b + 1])
                            nc.vector.tensor_tensor(a, a, t, aa)
                    nc.vector.tensor_tensor(a, a, w1rep[n], mm)
                    acc.append(a)

            # per-tap demod norm over (f, c)
            if n == 0:
                q = q0
            else:
                q = []
                for ct in range(CT):
                    scr = tmp_p.tile([P, TAPS, F], cdt, tag="tmp")
                    nc.scalar.activation(scr, acc[ct], ACT.Square)
                    qt = small_p.tile([P, TAPS], f32, tag="q")
                    nc.vector.tensor_reduce(qt, scr,
                                            axis=mybir.AxisListType.X, op=aa)
                    q.append(qt)
            qs = small_p.tile([P, TAPS], f32, tag="qs")
            nc.vector.tensor_tensor(qs, q[0], q[1], aa)
            ps_nrm = ps_small.tile([P, TAPS], f32, tag="sm")
            nc.tensor.matmul(ps_nrm, lhsT=ones_sb, rhs=qs,
                             start=True, stop=True)
            nall = small_p.tile([P, TAPS], f32, tag="nall")
            nc.vector.tensor_scalar_max(nall, ps_nrm, EPS)
            sq = small_p.tile([P, TAPS], f32, tag="sq")
            nc.scalar.activation(sq, nall, ACT.Sqrt)
            ninv = small_p.tile([P, TAPS], f32, tag="ninv")
            nc.vector.reciprocal(ninv, sq)
            if n == 0 and WARM_MM:
                for _ in range(4):
                    nc.tensor.matmul(ps_warm[:, :TAPS], lhsT=ones_sb,
                                     rhs=qs, start=True, stop=True)

            filt = []
            for ct in range(CT):
                fl = filtb_p.tile([P, TAPS, F], cdt, tag="filt")
                for tp in range(TAPS):
                    if tp % 2 == 0:
                        nc.vector.tensor_scalar_mul(
                            fl[:, tp, :], acc[ct][:, tp, :],
                            ninv[:, tp:tp + 1])
                    else:
                        nc.scalar.activation(fl[:, tp, :], acc[ct][:, tp, :],
                                             ACT.Copy,
                                             scale=ninv[:, tp:tp + 1])
                filt.append(fl)

            # conv in padded-flat coords: rhs slices are contiguous so the
            # PE streams at full rate; cols 64,65 of each row are garbage
            # and dropped by the strided output DMA.
            for ft in range(FT):
                osb = out_p.tile([P, OPIX], f32, tag="osb")
                for off, sz in CHUNKS:
                    ps = ps_conv.tile([P, 512], f32, tag="cv")
                    k = 0
                    for ct in range(CT):
                        for kh in range(KK):
                            for kw in range(KK):
                                rhs = xpt[ct][:, off + kh * WP + kw:
                                              off + kh * WP + kw + sz]
                                nc.tensor.matmul(
                                    ps[:, :sz],
                                    lhsT=filt[ct][:, kh * KK + kw,
                                                  ft * P:(ft + 1) * P],
                                    rhs=rhs,
                                    start=(k == 0), stop=(k == 2 * TAPS - 1))
                                k += 1
                    nc.scalar.activation(osb[:, off:off + sz], ps[:, :sz],
                                         ACT.Copy)
                ov = osb.rearrange("p (h w) -> p h w", w=WP)
                HS = 38
                nc.scalar.dma_start(out_d[n, ft * P:(ft + 1) * P, 0:HS, :],
                                    ov[:, 0:HS, 0:W])
                nc.scalar.dma_start(out_d[n, ft * P:(ft + 1) * P, HS:H, :],
                                    ov[:, HS:H, 0:W])

    nc.compile()
    return nc


def kernel(x, w, filter_bank, dense_fw_w, dense_fw_b, dense_mod_w, dense_mod_b):
    global LAST
    x = np.ascontiguousarray(np.asarray(x, dtype=np.float32))
    w = np.ascontiguousarray(np.asarray(w, dtype=np.float32))
    xdt = np.float32
    if USE_BF16:
        import ml_dtypes
        xdt = ml_dtypes.bfloat16
    NB = x.shape[0]
    xp_all = np.zeros((NB, CT, P, XLPAD), dtype=xdt)
    xv = xp_all[:, :, :, :XL].reshape(NB, CT, P, HP, WP)
    xv[:, :, :, 1:H + 1, 1:W + 1] = x.reshape(NB, CT, P, H, W)
    fb = np.asarray(filter_bank, dtype=np.float32)
    # [b, f, c, kh, kw] -> [b, c, (kh kw), f]
    bank_t = np.ascontiguousarray(
        np.transpose(fb, (0, 2, 3, 4, 1)).reshape(NF, C, TAPS, F))
    if USE_BF16:
        import ml_dtypes
        bank_t = bank_t.astype(ml_dtypes.bfloat16)

    trace = os.environ.get("KERNEL_TRACE", "") == "1"
    if trace:
        import types

        import concourse.bass_utils as bu
        bu.upload_artifacts = lambda tmpdir: tmpdir
        if "antenv.axon_hooks" not in sys.modules:
            from trn_agent_boot.trn_boot import _ntff_profile_via_ctypes
            hook = _ntff_profile_via_ctypes("/opt/axon/libaxon_pjrt.so")
            mod = types.ModuleType("antenv.axon_hooks")
            mod.get_axon_ntff_profile_hook = lambda: hook
            sys.modules["antenv.axon_hooks"] = mod

    nc = _build()
    in_maps = []
    for core in range(N_CORES):
        sl = slice(core * NS, (core + 1) * NS)
        w_t = np.ascontiguousarray(
            w[sl].reshape(NS, KO, P).transpose(2, 1, 0))
        ident = np.eye(P, dtype=xdt)
        in_maps.append({
            "ident": ident,
            "xp": np.ascontiguousarray(xp_all[sl]),
            "wv_t": w_t,
            "bank_t": bank_t,
            "fw_w": np.ascontiguousarray(np.asarray(dense_fw_w, np.float32).astype(xdt)),
            "fw_b": np.ascontiguousarray(np.asarray(dense_fw_b, np.float32)),
            "md_w": np.ascontiguousarray(np.asarray(dense_mod_w, np.float32).astype(xdt)),
            "md_b": np.ascontiguousarray(np.asarray(dense_mod_b, np.float32)),
        })
    kwargs = {}
    if trace:
        import tempfile
        base = os.environ.get("KERNEL_TRACE_DIR", "/tmp/ktrace")
        os.makedirs(base, exist_ok=True)
        tdir = tempfile.mkdtemp(dir=base)
        print(f"trace dir: {tdir}", flush=True)
        kwargs = dict(trace=True, tmpdir=tdir)
    LAST = run_bass_kernel_spmd(nc, in_maps, core_ids=list(range(N_CORES)),
                                **kwargs)
    return np.concatenate([LAST.results[i]["out"] for i in range(N_CORES)],
                          axis=0)



# revision 13
# speedup vs baseline: 1.1144x; 1.1144x over previous
"""AdaptiveConv Trainium2 kernel — 8-core SPMD, batch-sharded, 1-D Winograd.

Per full batch:
  x [16, 256, 64, 64] f32, w [16, 512] f32,
  filter_bank [8, 256, 256, 3, 3], dense_fw (512->8), dense_mod (512->256).
  fbw = softmax(w @ Wfw + bfw)                  [16, 8]
  filters = einsum('bfchw,nb->nfchw', bank, fbw)
  filters *= (w @ Wmod + bmod + 1)[n, f]
  norm[n,kh,kw] = sqrt(max(sum_{f,c} filters^2, 1e-8)); filters /= norm
  out[n] = conv2d_same(x[n], filters[n])        [16, 256, 64, 64]

Sharding: batch N=16 over 8 cores (2 samples each); params + the filter
bank replicated, bank loaded ONCE per core and kept SBUF-resident.

The conv runs as F(2,3) 1-D Winograd along W (direct over kh):
  per output pair (2t, 2t+1), with o[t]=x[w=2t-1], e[t]=x[w=2t]:
    xt0 = o[t]-o[t+1]   xt1 = e[t]+o[t+1]
    xt2 = o[t+1]-e[t]   xt3 = e[t]-e[t+1]
    ft0 = g0, ft1 = g0+g1+g2, ft2 = g0-g1+g2, ft3 = g2   (per kh, demodded)
    y_p = sum_{kh,c} ft_p * xt_p  (4 PSUM accumulations, K=768 each)
    even = 0.5*(y1+y2) + y0 ; odd = 0.5*(y1-y2) - y3
  => 24 matmuls per 16-row chunk instead of 36: 2/3 the PE work, and no
  garbage columns (pairs tile W exactly), so PE conv time drops ~35%.

x is staged host-side as zero-padded, w-deinterleaved odd/even planes so
every DVE transform op reads unit-stride bf16 (2x mode). Mix of the bank
runs on the PE as diagonal matmuls for sample 0 (both c-tiles) and
sample 1 c-tile 0 (chasing the bank DMA), with sample 1 c-tile 1 mixed
on DVE under sample 0's conv. Outputs drain per 16-row chunk straight
into compact [128, 16*64] f32 tiles DMA'd with full-size descriptors.
"""

import os
import sys

import numpy as np

if "/opt/trn_rl_repo" not in sys.path:
    sys.path.insert(0, "/opt/trn_rl_repo")

import concourse.bacc as bacc_mod
import concourse.mybir as mybir
import concourse.tile as tile
from concourse.bass_utils import run_bass_kernel_spmd

N_CORES = 8
NS = 2            # samples per core
C = 256
F = 256
H = W = 64
KK = 3
TAPS = KK * KK    # 9
NF = 8
WD = 512
P = 128
CT = C // P       # 2 c tiles
FT = F // P       # 2 f tiles
KO = WD // P      # 4 contraction tiles for the dense layers
HP = H + 2        # 66 rows in the winograd input (h = -1..64)
NT = W // 2       # 32 output pairs per row
TPL = HP * NT     # 2112 positions per transform plane
EO = NT + 2       # 34 cols per deinterleaved odd/even plane
HCH = 16          # output rows per conv chunk
CHN = H // HCH    # 4 chunks per (sample, f-tile)
CHL = HCH * NT    # 512 positions per chunk
MIX_CHUNKS = [(0, 512), (512, 512), (1024, 512), (1536, 512), (2048, 256)]
EPS = 1e-8

USE_BF16 = os.environ.get("KERNEL_F32", "") != "1"
WARM_MM = int(os.environ.get("KERNEL_WARM_MM", "2"))

LAST = None       # BassKernelResults of the most recent run (for test.py)


def _build():
    f32 = mybir.dt.float32
    cdt = mybir.dt.bfloat16 if USE_BF16 else f32

    nc = bacc_mod.Bacc()
    xdi_d = nc.declare_dram_parameter("xdi", [NS, CT, P, 2, HP, EO], cdt,
                                      isOutput=False)
    w_d = nc.declare_dram_parameter("wv_t", [P, KO, NS], f32, isOutput=False)
    bank_d = nc.declare_dram_parameter("bank_t", [NF, C, TAPS, F], cdt,
                                       isOutput=False)
    fww_d = nc.declare_dram_parameter("fw_w", [WD, NF], cdt, isOutput=False)
    fwb_d = nc.declare_dram_parameter("fw_b", [NF], f32, isOutput=False)
    mdw_d = nc.declare_dram_parameter("md_w", [WD, F], cdt, isOutput=False)
    mdb_d = nc.declare_dram_parameter("md_b", [F], f32, isOutput=False)
    ident_d = nc.declare_dram_parameter("ident", [P, P], cdt, isOutput=False)
    out_d = nc.declare_dram_parameter("out", [NS, F, H, W], f32, isOutput=True)

    mm = mybir.AluOpType.mult
    aa = mybir.AluOpType.add
    ss = mybir.AluOpType.subtract
    ACT = mybir.ActivationFunctionType

    with tile.TileContext(nc) as tc, \
         tc.tile_pool(name="const", bufs=1) as const_p, \
         tc.tile_pool(name="small", bufs=2) as small_p, \
         tc.tile_pool(name="bcast", bufs=2) as bc_p, \
         tc.tile_pool(name="diag", bufs=16) as diag_p, \
         tc.tile_pool(name="bank", bufs=16) as bk_p, \
         tc.tile_pool(name="xdi", bufs=2) as xdi_p, \
         tc.tile_pool(name="xt", bufs=8) as xt_p, \
         tc.tile_pool(name="acc", bufs=3) as acc_p, \
         tc.tile_pool(name="tmp", bufs=2) as tmp_p, \
         tc.tile_pool(name="filt", bufs=2) as filt_p, \
         tc.tile_pool(name="f12", bufs=2) as f12_p, \
         tc.tile_pool(name="scr", bufs=2) as scr_p, \
         tc.tile_pool(name="fs", bufs=2) as fs_p, \
         tc.tile_pool(name="outs", bufs=2) as out_p, \
         tc.tile_pool(name="pscv", bufs=6, space="PSUM") as ps_cv, \
         tc.tile_pool(name="pssm", bufs=1, space="PSUM") as ps_sm:

        # preload ACT tables (Exp/Square/Sqrt) with a self-contained tile
        tblw = small_p.tile([P, 2], f32, name="tblw")
        nc.vector.memset(tblw, 0.5)
        nc.scalar.activation(tblw, tblw, ACT.Exp)
        nc.scalar.activation(tblw, tblw, ACT.Square)
        nc.scalar.activation(tblw, tblw, ACT.Sqrt)

        # ---- bank DMA first (the long pole): ct-major, b-inner ----------
        bkr = [[None] * NF for _ in range(CT)]
        for ct in range(CT):
            for b in range(NF):
                bk = bk_p.tile([P, TAPS, F], cdt, tag="bk", name=f"bk{ct}_{b}")
                nc.sync.dma_start(bk, bank_d[b, ct * P:(ct + 1) * P, :, :])
                bkr[ct][b] = bk

        # ---- small parameter loads (scalar queue) -----------------------
        wt = const_p.tile([P, KO, NS], f32)
        nc.scalar.dma_start(wt, w_d[:, :, :])
        fww = const_p.tile([P, KO, NF], cdt)
        nc.scalar.dma_start(fww, fww_d[:, :].rearrange("(ko p) f -> p ko f", p=P))
        mdw = const_p.tile([P, KO, F], cdt)
        nc.scalar.dma_start(mdw, mdw_d[:, :].rearrange("(ko p) f -> p ko f", p=P))
        fwb_bc = const_p.tile([P, NF], f32)
        nc.scalar.dma_start(fwb_bc, fwb_d[:][None, :].to_broadcast((P, NF)))
        mdb_bc = const_p.tile([P, F], f32)
        nc.scalar.dma_start(mdb_bc, mdb_d[:][None, :].to_broadcast((P, F)))
        ident = const_p.tile([P, P], cdt)
        nc.scalar.dma_start(ident, ident_d[:, :])

        # ---- x sample 0 (gpsimd queue) ----------------------------------
        xq0 = []
        for ct in range(CT):
            xq = xdi_p.tile([P, 2, HP, EO], cdt, tag="xdi", name=f"xq0_{ct}")
            nc.gpsimd.dma_start(xq, xdi_d[0, ct, :, :, :, :])
            xq0.append(xq)

        # ---- dense heads, replicated across all 128 partitions ----------
        fbw_bc = []
        w1rep = []
        for n in range(NS):
            wrep = bc_p.tile([P, KO, P], cdt, tag="wrep")
            nc.vector.tensor_copy(
                wrep, wt[:, :, n][:, :, None].to_broadcast((P, KO, P)))
            ps_l = ps_sm.tile([P, NF], f32, tag="sm")
            for ko in range(KO):
                nc.tensor.matmul(ps_l, lhsT=wrep[:, ko, :],
                                 rhs=fww[:, ko, :],
                                 start=(ko == 0), stop=(ko == KO - 1))
            logits = small_p.tile([P, NF], f32, tag="logits")
            nc.vector.tensor_tensor(logits, ps_l, fwb_bc, aa)
            # softmax denominator cancels through the per-tap demod norm
            fb = bc_p.tile([P, NF], f32, tag="fbw_bc")
            nc.scalar.activation(fb, logits, ACT.Exp)
            fbw_bc.append(fb)

            ps_sc = ps_sm.tile([P, F], f32, tag="sm")
            for ko in range(KO):
                nc.tensor.matmul(ps_sc, lhsT=wrep[:, ko, :], rhs=mdw[:, ko, :],
                                 start=(ko == 0), stop=(ko == KO - 1))
            w1 = bc_p.tile([P, F], cdt, tag="w1_bc")
            nc.vector.scalar_tensor_tensor(w1, ps_sc, 1.0, mdb_bc, aa, aa)
            w1r = bc_p.tile([P, TAPS, F], cdt, tag="w1rep")
            nc.vector.tensor_copy(
                w1r, w1[:, None, :].to_broadcast((P, TAPS, F)))
            w1rep.append(w1r)

        ps_warm = ps_sm.tile([P, 256], f32, tag="warm")
        ones_sb = const_p.tile([P, P], f32)
        nc.vector.memset(ones_sb, 1.0)

        # diag(fbw_b) weight tiles for the PE mixes (samples 0 and 1)
        fbwI = [[], []]
        for n in range(NS):
            for b in range(NF):
                dg = diag_p.tile([P, P], cdt, tag="diag", name=f"dg{n}_{b}")
                nc.vector.tensor_scalar_mul(dg, ident, fbw_bc[n][:, b:b + 1])
                fbwI[n].append(dg)
                if WARM_MM and n == 0:
                    for _ in range(2):
                        nc.tensor.matmul(ps_warm[:, :P], lhsT=dg, rhs=dg,
                                         start=True, stop=True)

        # ---- x-tilde transform for sample 0 (DVE, unit-stride bf16) -----
        # xq[ct] planes: [:,0]=odd (w=2t-1), [:,1]=even (w=2t)
        def xt_transform(xq, pool, tag):
            xts = []
            for ct in range(CT):
                o = xq[ct][:, 0, :, :]
                e = xq[ct][:, 1, :, :]
                pl = []
                for p in range(4):
                    t = pool.tile([P, HP, NT], cdt, tag=tag,
                                  name=f"xt{tag}{ct}_{p}")
                    pl.append(t)
                nc.vector.tensor_tensor(pl[0], o[:, :, 0:NT], o[:, :, 1:NT + 1], ss)
                nc.vector.tensor_tensor(pl[1], e[:, :, 0:NT], o[:, :, 1:NT + 1], aa)
                nc.vector.tensor_tensor(pl[2], o[:, :, 1:NT + 1], e[:, :, 0:NT], ss)
                nc.vector.tensor_tensor(pl[3], e[:, :, 0:NT], e[:, :, 1:NT + 1], ss)
                xts.append(pl)
            return xts

        xt0 = xt_transform(xq0, xt_p, "xt")

        # ---- mix phase ---------------------------------------------------
        # PE diag-matmul mix for (n0,ct0) chasing DMA, then (n1,ct0) from
        # resident tiles, then (n0,ct1) chasing. (n1,ct1) runs on DVE later.
        acc = [[None] * CT for _ in range(NS)]
        scrq = [[None] * CT for _ in range(NS)]
        qt = [[None] * CT for _ in range(NS)]

        def pe_mix(n, ct):
            a = acc_p.tile([P, TAPS, F], cdt, tag="acc", name=f"acc{n}_{ct}")
            af = a.rearrange("p t f -> p (t f)")
            w1f = w1rep[n].rearrange("p t f -> p (t f)")
            pss = [ps_cv.tile([P, 512], f32, tag="cv", name=f"mix{n}{ct}_{ci}")
                   for ci in range(len(MIX_CHUNKS))]
            for b in range(NF):
                bf = bkr[ct][b].rearrange("p t f -> p (t f)")
                for ci, (off, csz) in enumerate(MIX_CHUNKS):
                    nc.tensor.matmul(pss[ci][:, :csz], lhsT=fbwI[n][b],
                                     rhs=bf[:, off:off + csz],
                                     start=(b == 0), stop=(b == NF - 1))
                if WARM_MM and n == 0 and ct == 0:
                    nc.tensor.matmul(ps_warm, lhsT=bf[:, 0:P], rhs=bf[:, 0:256],
                                     start=True, stop=True)
            # drains: modulate by (scales+1) as the psums close (DVE)
            for ci, (off, csz) in enumerate(MIX_CHUNKS):
                nc.vector.tensor_tensor(af[:, off:off + csz], pss[ci][:, :csz],
                                        w1f[:, off:off + csz], mm)
            acc[n][ct] = a
            # squares for the demod norm (scalar), reduce later
            scr = tmp_p.tile([P, TAPS, F], cdt, tag="tmp", name=f"sq{n}_{ct}")
            nc.scalar.activation(scr, a, ACT.Square)
            scrq[n][ct] = scr

        pe_mix(0, 0)
        pe_mix(1, 0)
        pe_mix(0, 1)

        # reduce the squares for sample 0 (DVE; [P,9] per ct)
        for ct in range(CT):
            q = small_p.tile([P, TAPS], f32, tag="q", name=f"q0_{ct}")
            nc.vector.tensor_reduce(q, scrq[0][ct],
                                    axis=mybir.AxisListType.X, op=aa)
            qt[0][ct] = q

        def norm_chain(n):
            qs = small_p.tile([P, TAPS], f32, tag="qs", name=f"qs{n}")
            nc.vector.tensor_tensor(qs, qt[n][0], qt[n][1], aa)
            ps_nrm = ps_sm.tile([P, TAPS], f32, tag="sm", name=f"psn{n}")
            nc.tensor.matmul(ps_nrm, lhsT=ones_sb, rhs=qs,
                             start=True, stop=True)
            nall = small_p.tile([P, TAPS], f32, tag="nall", name=f"na{n}")
            nc.vector.tensor_scalar_max(nall, ps_nrm, EPS)
            sq = small_p.tile([P, TAPS], f32, tag="sq", name=f"sv{n}")
            nc.scalar.activation(sq, nall, ACT.Sqrt)
            ninv = small_p.tile([P, TAPS], f32, tag="ninv", name=f"ni{n}")
            nc.vector.reciprocal(ninv, sq)
            return ninv

        ninv0 = norm_chain(0)

        # ---- demod + winograd filter transform for sample 0 -------------
        # filt holds the demodded taps (ft0 = tap kw0, ft3 = tap kw2);
        # f12 holds ft1 = g0+g1+g2 and ft2 = g0-g1+g2 per kh.
        def filter_transform(n, ninv, fpool, f12pool, veng, tagsuf=""):
            filt, f12 = [], []
            for ct in range(CT):
                fl = fpool.tile([P, TAPS, F], cdt, tag="filt" + tagsuf,
                                name=f"fl{n}_{ct}")
                fx = f12pool.tile([P, 2, KK, F], cdt, tag="f12" + tagsuf,
                                  name=f"fx{n}_{ct}")
                filt.append(fl)
                f12.append(fx)
            for kh in range(KK):
                for ct in range(CT):
                    fl, fx, a = filt[ct], f12[ct], acc[n][ct]
                    for kw in range(KK):
                        tp = kh * KK + kw
                        if kw == 1:
                            nc.scalar.activation(fl[:, tp, :], a[:, tp, :],
                                                 ACT.Copy,
                                                 scale=ninv[:, tp:tp + 1])
                        else:
                            veng.tensor_scalar_mul(fl[:, tp, :], a[:, tp, :],
                                                   ninv[:, tp:tp + 1])
                    s = fs_p.tile([P, F], cdt, tag="fs", name=f"fs{n}_{ct}_{kh}")
                    veng.tensor_tensor(s, fl[:, kh * KK, :],
                                       fl[:, kh * KK + 2, :], aa)
                    veng.tensor_tensor(fx[:, 0, kh, :], s,
                                       fl[:, kh * KK + 1, :], aa)
                    veng.tensor_tensor(fx[:, 1, kh, :], s,
                                       fl[:, kh * KK + 1, :], ss)
            return filt, f12

        filt0, f120 = filter_transform(0, ninv0, filt_p, f12_p, nc.vector)

        if WARM_MM:
            for _ in range(4):
                nc.tensor.matmul(ps_warm[:, :TAPS], lhsT=ones_sb,
                                 rhs=qt[0][0], start=True, stop=True)

        # ---- x sample 1 DMA (gpsimd queue), transform later on DVE ------
        # allocate sample-1 acc (bank slot 0: free after (n1,ct0) PE mix)
        acc11 = bk_p.tile([P, TAPS, F], cdt, tag="bk", name="acc1_1")
        xq1 = []
        for ct in range(CT):
            xq = xdi_p.tile([P, 2, HP, EO], cdt, tag="xdi", name=f"xq1_{ct}")
            nc.gpsimd.dma_start(xq, xdi_d[1, ct, :, :, :, :])
            xq1.append(xq)

        # ---- conv + drains, interleaved with sample-1 prep --------------
        def conv_sample(n, filt, f12, xts, vwork):
            """vwork: list of callables issuing one DVE work item each,
            interleaved between chunk drains."""
            wi = 0
            for ft in range(FT):
                fsl = slice(ft * P, (ft + 1) * P)
                for hc in range(CHN):
                    pss = [ps_cv.tile([P, CHL], f32, tag="cv",
                                      name=f"cv{n}{ft}{hc}_{p}")
                           for p in range(4)]
                    for p in (1, 2, 0, 3):
                        k = 0
                        for kh in range(KK):
                            for ct in range(CT):
                                if p == 0:
                                    lhs = filt[ct][:, kh * KK, fsl]
                                elif p == 3:
                                    lhs = filt[ct][:, kh * KK + 2, fsl]
                                else:
                                    lhs = f12[ct][:, p - 1, kh, fsl]
                                xf = xts[ct][p].rearrange("p h t -> p (h t)")
                                off = (hc * HCH + kh) * NT
                                nc.tensor.matmul(
                                    pss[p][:, :], lhsT=lhs,
                                    rhs=xf[:, off:off + CHL],
                                    start=(k == 0), stop=(k == 2 * KK - 1))
                                k += 1
                    osb = out_p.tile([P, HCH, W], f32, tag="osb",
                                     name=f"osb{n}{ft}{hc}")
                    ov = osb.rearrange("p h (t two) -> p h t two", two=2)
                    av = scr_p.tile([P, HCH, NT], f32, tag="a",
                                    name=f"a{n}{ft}{hc}")
                    sv = scr_p.tile([P, HCH, NT], f32, tag="s",
                                    name=f"s{n}{ft}{hc}")
                    dv = scr_p.tile([P, HCH, NT], f32, tag="d",
                                    name=f"d{n}{ft}{hc}")
                    p1 = pss[1].rearrange("p (h t) -> p h t", t=NT)
                    p2 = pss[2].rearrange("p (h t) -> p h t", t=NT)
                    p0 = pss[0].rearrange("p (h t) -> p h t", t=NT)
                    p3 = pss[3].rearrange("p (h t) -> p h t", t=NT)
                    # one-PSUM-input-per-op drain:
                    #   a = 0.5*y1 (ACT); s = 0.5*y2 + a; d = -0.5*y2 + a
                    #   even = s + y0 ; odd = -y3 + d
                    nc.scalar.activation(av, p1, ACT.Copy, scale=0.5)
                    nc.vector.scalar_tensor_tensor(sv, p2, 0.5, av, mm, aa)
                    nc.vector.scalar_tensor_tensor(dv, p2, -0.5, av, mm, aa)
                    nc.vector.tensor_tensor(ov[:, :, :, 0], sv, p0, aa)
                    nc.vector.scalar_tensor_tensor(ov[:, :, :, 1], p3, -1.0,
                                                   dv, mm, aa)
                    nc.sync.dma_start(
                        out_d[n, fsl, hc * HCH:(hc + 1) * HCH, :], osb)
                    for _ in range(2):
                        if wi < len(vwork):
                            vwork[wi]()
                            wi += 1
            while wi < len(vwork):
                vwork[wi]()
                wi += 1

        # sample-1 DVE work items, fed between sample-0 chunk drains:
        # mix (n1,ct1) via fused stt ops, then xt transform of sample 1.
        xt1 = [[None] * 4 for _ in range(CT)]

        def mk_mix1(b):
            def fn():
                bf = bkr[1][b].rearrange("p t f -> p (t f)")
                a1 = acc11.rearrange("p t f -> p (t f)")
                if b == 0:
                    nc.vector.tensor_scalar_mul(a1, bf, fbw_bc[1][:, 0:1])
                else:
                    nc.vector.scalar_tensor_tensor(
                        a1, bf, fbw_bc[1][:, b:b + 1], a1, mm, aa)
                if b == NF - 1:
                    w1f = w1rep[1].rearrange("p t f -> p (t f)")
                    nc.vector.tensor_tensor(a1, a1, w1f, mm)
                    acc[1][1] = acc11
                    scr = bk_p.tile([P, TAPS, F], cdt, tag="bk", name="sq1_1")
                    nc.scalar.activation(scr, acc11, ACT.Square)
                    scrq[1][1] = scr
                    q = small_p.tile([P, TAPS], f32, tag="q", name="q1_1")
                    nc.vector.tensor_reduce(q, scr,
                                            axis=mybir.AxisListType.X, op=aa)
                    qt[1][1] = q
            return fn

        def mk_xt1(ct, p):
            def fn():
                o = xq1[ct][:, 0, :, :]
                e = xq1[ct][:, 1, :, :]
                t = bk_p.tile([P, HP, NT], cdt, tag="bk", name=f"xt1{ct}_{p}")
                if p == 0:
                    nc.vector.tensor_tensor(t, o[:, :, 0:NT], o[:, :, 1:NT + 1], ss)
                elif p == 1:
                    nc.vector.tensor_tensor(t, e[:, :, 0:NT], o[:, :, 1:NT + 1], aa)
                elif p == 2:
                    nc.vector.tensor_tensor(t, o[:, :, 1:NT + 1], e[:, :, 0:NT], ss)
                else:
                    nc.vector.tensor_tensor(t, e[:, :, 0:NT], e[:, :, 1:NT + 1], ss)
                xt1[ct][p] = t
            return fn

        # square+reduce for (n1,ct0) happened on PE mix path: do it now
        scr10 = tmp_p.tile([P, TAPS, F], cdt, tag="tmp", name="sq1_0")
        nc.scalar.activation(scr10, acc[1][0], ACT.Square)
        scrq[1][0] = scr10
        q10 = small_p.tile([P, TAPS], f32, tag="q", name="q1_0")
        nc.vector.tensor_reduce(q10, scr10, axis=mybir.AxisListType.X, op=aa)
        qt[1][0] = q10

        vwork = [mk_mix1(b) for b in range(NF)]
        vwork += [mk_xt1(ct, p) for ct in range(CT) for p in range(4)]

        conv_sample(0, filt0, f120, xt0, vwork)

        # ---- sample 1 norm / demod / filter transform -------------------
        ninv1 = norm_chain(1)
        filt1, f121 = [], []
        for ct in range(CT):
            fl = bk_p.tile([P, TAPS, F], cdt, tag="bk", name=f"fl1_{ct}")
            fx = bk_p.tile([P, 2, KK, F], cdt, tag="bk", name=f"fx1_{ct}")
            filt1.append(fl)
            f121.append(fx)
        for kh in range(KK):
            for ct in range(CT):
                fl, fx, a = filt1[ct], f121[ct], acc[1][ct]
                for kw in range(KK):
                    tp = kh * KK + kw
                    nc.scalar.activation(fl[:, tp, :], a[:, tp, :],
                                         ACT.Copy, scale=ninv1[:, tp:tp + 1])
                s = fs_p.tile([P, F], cdt, tag="fs", name=f"fs1_{ct}_{kh}")
                nc.gpsimd.tensor_tensor(s, fl[:, kh * KK, :],
                                        fl[:, kh * KK + 2, :], aa)
                nc.gpsimd.tensor_tensor(fx[:, 0, kh, :], s,
                                        fl[:, kh * KK + 1, :], aa)
                nc.gpsimd.tensor_tensor(fx[:, 1, kh, :], s,
                                        fl[:, kh * KK + 1, :], ss)

        conv_sample(1, filt1, f121, xt1, [])

    nc.compile()
    return nc


def kernel(x, w, filter_bank, dense_fw_w, dense_fw_b, dense_mod_w, dense_mod_b):
    global LAST
    x = np.ascontiguousarray(np.asarray(x, dtype=np.float32))
    w = np.ascontiguousarray(np.asarray(w, dtype=np.float32))
    xdt = np.float32
    if USE_BF16:
        import ml_dtypes
        xdt = ml_dtypes.bfloat16
    NB = x.shape[0]
    # deinterleaved, padded odd/even planes:
    #   odd[t]  = x[w=2t-1] (t=0..33, zeros at w=-1 and w=65)
    #   even[t] = x[w=2t]   (t=0..33, zeros at w=64.. )
    # rows r=0..65 map to h=r-1 with zero padding at h=-1, 64.
    xr = x.reshape(NB, CT, P, H, W)
    xdi_all = np.zeros((NB, CT, P, 2, HP, EO), dtype=xdt)
    xdi_all[:, :, :, 0, 1:H + 1, 1:NT + 1] = xr[:, :, :, :, 1::2]
    xdi_all[:, :, :, 1, 1:H + 1, 0:NT] = xr[:, :, :, :, 0::2]
    fb = np.asarray(filter_bank, dtype=np.float32)
    # [b, f, c, kh, kw] -> [b, c, (kh kw), f]
    bank_t = np.ascontiguousarray(
        np.transpose(fb, (0, 2, 3, 4, 1)).reshape(NF, C, TAPS, F))
    if USE_BF16:
        import ml_dtypes
        bank_t = bank_t.astype(ml_dtypes.bfloat16)

    trace = os.environ.get("KERNEL_TRACE", "") == "1"
    if trace:
        import types

        import concourse.bass_utils as bu
        bu.upload_artifacts = lambda tmpdir: tmpdir
        if "antenv.axon_hooks" not in sys.modules:
            from trn_agent_boot.trn_boot import _ntff_profile_via_ctypes
            hook = _ntff_profile_via_ctypes("/opt/axon/libaxon_pjrt.so")
            mod = types.ModuleType("antenv.axon_hooks")
            mod.get_axon_ntff_profile_hook = lambda: hook
            sys.modules["antenv.axon_hooks"] = mod

    nc = _build()
    in_maps = []
    for core in range(N_CORES):
        sl = slice(core * NS, (core + 1) * NS)
        w_t = np.ascontiguousarray(
            w[sl].reshape(NS, KO, P).transpose(2, 1, 0))
        ident = np.eye(P, dtype=xdt)
        in_maps.append({
            "ident": ident,
            "xdi": np.ascontiguousarray(xdi_all[sl]),
            "wv_t": w_t,
            "bank_t": bank_t,
            "fw_w": np.ascontiguousarray(np.asarray(dense_fw_w, np.float32).astype(xdt)),
            "fw_b": np.ascontiguousarray(np.asarray(dense_fw_b, np.float32)),
            "md_w": np.ascontiguousarray(np.asarray(dense_mod_w, np.float32).astype(xdt)),
            "md_b": np.ascontiguousarray(np.asarray(dense_mod_b, np.float32)),
        })
    kwargs = {}
    if trace:
        import tempfile
        base = os.environ.get("KERNEL_TRACE_DIR", "/tmp/ktrace")
        os.makedirs(base, exist_ok=True)
        tdir = tempfile.mkdtemp(dir=base)
        print(f"trace dir: {tdir}", flush=True)
        kwargs = dict(trace=True, tmpdir=tdir)
    LAST = run_bass_kernel_spmd(nc, in_maps, core_ids=list(range(N_CORES)),
                                **kwargs)
    return np.concatenate([LAST.results[i]["out"] for i in range(N_CORES)],
                          axis=0)
